# revision 58
# baseline (speedup 1.0000x reference)
# Trainium2 Bass kernel for nn_AutoformerDecoderLayer (B=8,L=1024,D=512,DFF=2048,H=8,DK=64)
# Strategy: data-parallel over batch B across 8 NeuronCores (zero collectives).
# Each core runs the full decoder layer on one [1024, 512] batch element.
#
# v4 design notes (on top of v3):
#  - q/k drains descale by 1/(SX*SW) so exp scale is 1/8 (natural units);
#    the ALiBi bias is preloaded as 8*bias split hi+lo into two fp8 chunks
#    and injected with ONE DoubleRow matmul per 4-head super-group (half the
#    PE cost of the v3 bf16 ident preloads).
#  - Scores run in 2 super-groups of 4 heads per k-tile ([128,1024] PSUM
#    tiles) so exp is 2 activations per k-tile instead of 4.
#  - mov drains write bf16; LN normalize ops run as DVE tensor_scalar in 4x
#    mode (all-SBUF, 16-bit); the bf16->fp8 quant runs on the Scalar engine
#    (idle in LN windows) as one op per l-half.
#  - LN transposes are direct SBUF->SBUF 128x128 DMA transposes (no DRAM
#    round trip).
#  - LN half 0 (tiles 0-3) is emitted INSIDE the attention loop (stats for
#    tiles 0-3 are ready at kt=5), so the next phase's projections start
#    with at most the half-1 latency; CA k/v projections are emitted right
#    after the SA loop to keep PE busy during LN1 half 1.
#  - LN3 halves are emitted inside the FFN loop; output is bf16 (converted
#    to f32 on host).
#  - Q projections are hoisted out of the attention core into their own
#    PSUM scope.
import sys

sys.path.insert(0, "/opt/trn_rl_repo")

from contextlib import ExitStack

import numpy as np
import ml_dtypes

B, L, D, DFF, H, DK = 8, 1024, 512, 2048, 8, 64
KSZ = 25
PAD = KSZ // 2
EPS = 1e-5
NLT = L // 128      # 8 l-tiles
NDC = D // 128      # 4 d-chunks
NFT = DFF // 128    # 16 dff tiles
BF16 = ml_dtypes.bfloat16
F8 = ml_dtypes.float8_e4m3

SW = 256.0    # fp8 weight scale (qkv / W1)
SX = 16.0     # fp8 activation scale
SW2 = 64.0    # FFN2 weight scale == r3 residual scale (LN3 absorbs it)
DSC = 1.0 / (SX * SW)   # q/k drain descale -> natural units
EXP_SCALE = 1.0 / 8.0   # 1/sqrt(DK)
WIN = 256     # per-k-tile q window; starts at 128*kt - 64
_CACHE = {}


def _host_constants():
    # Bias for the win-256 window: k = 128*kt + i, q = 128*kt-64 + c.
    # Preloaded into PSUM as 8*bias = hi + lo (two fp8 chunks), duplicated
    # across the 4 heads of a super-group: d_cat8 [128, 2*4*WIN].
    i = np.arange(128)[:, None].astype(np.float64)
    c = np.arange(WIN)[None, :].astype(np.float64)
    b8 = 8.0 * (-0.1 * np.abs(c - 64.0 - i))          # [128, 256] in [-154, 0]
    hi = b8.astype(F8)
    lo = (b8 - hi.astype(np.float64)).astype(F8)
    d_cat8 = np.concatenate([np.tile(hi, (1, 4)), np.tile(lo, (1, 4))], axis=1)

    # Moving-average matrix A[lo, li] = 1/25 iff |lo-li| <= 12, packed into
    # the exact a_sb SBUF layout: 22 banded [128, 128] blocks side by side.
    lo_i = np.arange(L)[:, None]
    li = np.arange(L)[None, :]
    A = ((np.abs(lo_i - li) <= PAD).astype(np.float64) / KSZ).astype(np.float32)
    blocks = []
    for t in range(NLT):
        for j in range(max(0, t - 1), min(NLT, t + 2)):
            blocks.append(A[128 * j:128 * (j + 1), 128 * t:128 * (t + 1)])
    a_strip = np.concatenate(blocks, axis=1)  # [128, 22*128]
    return d_cat8, a_strip


def _build_program(reps=1):
    """Build (and cache) the single-core Bass program + compile it.

    reps>1 repeats the whole layer body (timing calibration only)."""
    key = ("nc", reps)
    if key in _CACHE:
        return _CACHE[key]

    import concourse.tile as tile
    import concourse.mybir as mybir
    from concourse import bacc
    from concourse.bass import AP as BassAP

    f32 = mybir.dt.float32
    f32r = mybir.dt.float32r
    bf16 = mybir.dt.bfloat16
    fp8 = mybir.dt.float8e4
    AF = mybir.ActivationFunctionType
    ALU = mybir.AluOpType
    DR = mybir.MatmulPerfMode.DoubleRow

    nc = bacc.Bacc("TRN2", target_bir_lowering=False, debug=False)

    # ---------------- DRAM parameters (per-core shapes) ----------------
    def din(name, shape, dt=f32):
        return nc.dram_tensor(name, list(shape), dt, kind="ExternalInput").ap()

    xT8_d = din("xT8", (D, L), fp8)      # x.T * SX
    encT8_d = din("encT8", (D, L), fp8)  # enc.T * SX
    x_bf_d = din("x_bf", (L, D), bf16)   # residual base
    wq_sa8 = din("wq_sa8", (D, D), fp8)  # W.T * SW
    wk_sa8 = din("wk_sa8", (D, D), fp8)
    wv_sa8 = din("wv_sa8", (D, D), fp8)
    wo_sa = din("wo_sa", (D, D), bf16)
    wq_ca8 = din("wq_ca8", (D, D), fp8)  # W.T * SW
    wk_ca8 = din("wk_ca8", (D, D), fp8)
    wv_ca8 = din("wv_ca8", (D, D), fp8)
    wo_ca = din("wo_ca", (D, D), bf16)
    w18 = din("w18", (D, DFF), fp8)      # W1.T * SW
    w28 = din("w28", (DFF, D), fp8)      # W2.T * SW2
    d_cat8_d = din("d_cat8", (128, 2 * 4 * WIN), fp8)
    a_strip_d = din("a_strip", (128, 22 * 128), f32r)
    ident8_d = din("ident8", (128, 2 * 128), fp8)
    out_d = nc.dram_tensor("out", [L, D], bf16, kind="ExternalOutput").ap()

    with tile.TileContext(nc) as tc, ExitStack() as ctx:
        persist = ctx.enter_context(tc.tile_pool(name="persist", bufs=1))
        streams = ctx.enter_context(tc.tile_pool(name="streams", bufs=2))
        movp = ctx.enter_context(tc.tile_pool(name="movp", bufs=1))
        srcp8 = ctx.enter_context(tc.tile_pool(name="srcp8", bufs=2))
        srcp16 = ctx.enter_context(tc.tile_pool(name="srcp16", bufs=1))
        bfbuf = ctx.enter_context(tc.tile_pool(name="bfbuf", bufs=1))
        nbf_p = ctx.enter_context(tc.tile_pool(name="nbf_p", bufs=3))
        expp = ctx.enter_context(tc.tile_pool(name="expp", bufs=4))
        stats_p = ctx.enter_context(tc.tile_pool(name="stats", bufs=2))
        small = ctx.enter_context(tc.tile_pool(name="small", bufs=4))

        # ---------- tiny constants ----------
        d_cat8 = persist.tile([128, 2 * 4 * WIN], fp8, tag="d_cat8")
        nc.sync.dma_start(out=d_cat8, in_=d_cat8_d)
        ident8 = persist.tile([128, 2 * 128], fp8, tag="ident8")
        nc.sync.dma_start(out=ident8, in_=ident8_d)
        eps_sb = persist.tile([128, 1], f32, tag="eps")
        nc.vector.memset(eps_sb, EPS)
        eps3_sb = persist.tile([128, 1], f32, tag="eps3")
        nc.vector.memset(eps3_sb, EPS * SW2 * SW2)
        warm = persist.tile([128, 1], f32, tag="warm")
        nc.scalar.activation(out=warm, in_=eps_sb, func=AF.Exp)
        nc.scalar.activation(out=warm, in_=eps_sb, func=AF.Sqrt)
        nc.scalar.activation(out=warm, in_=eps_sb, func=AF.Gelu)

        a_sb = persist.tile([128, 22 * 128], f32r, tag="a_sb")
        a_blocks = {}
        bi = 0
        for t in range(NLT):
            for j in range(max(0, t - 1), min(NLT, t + 2)):
                a_blocks[(t, j)] = bi
                bi += 1
        a_loaded = [False]

        def ensure_a():
            if not a_loaded[0]:
                a_loaded[0] = True
                # two DMAs so the first tails' blocks land earlier
                nc.gpsimd.dma_start(out=a_sb[:, :11 * 128],
                                    in_=a_strip_d[:, :11 * 128])
                nc.gpsimd.dma_start(out=a_sb[:, 11 * 128:],
                                    in_=a_strip_d[:, 11 * 128:])

        def bcast64(ap):
            """[128, n] AP -> [128, n, 64] stride-0 broadcast AP."""
            return BassAP(ap.tensor, ap.offset, list(ap.ap) + [[0, 64]])

        # ================= helpers =================
        def load_w8(wpool, dram_ap, tag, eng=None):
            t = wpool.tile([128, NDC * 512], fp8, tag=tag)
            (eng or nc.sync).dma_start(
                out=t.rearrange("p (c n) -> p c n", c=NDC),
                in_=dram_ap.rearrange("(c p) n -> p c n", p=128),
            )
            return t

        def load_w16(wpool, dram_ap, tag, eng=None):
            t = wpool.tile([128, NDC * 512], bf16, tag=tag)
            (eng or nc.sync).dma_start(
                out=t.rearrange("p (c n) -> p c n", c=NDC),
                in_=dram_ap.rearrange("(c p) n -> p c n", p=128),
            )
            return t

        def load_srcT8(dram_ap, tag, eng=None):
            t = srcp8.tile([128, NDC * 1024], fp8, tag=tag)
            for lh in range(2):  # l-halves so the first projections start early
                (eng or nc.sync).dma_start(
                    out=t.rearrange("p (c l) -> p c l", c=NDC)[
                        :, :, 512 * lh:512 * (lh + 1)],
                    in_=dram_ap.rearrange("(c p) l -> p c l", p=128)[
                        :, :, 512 * lh:512 * (lh + 1)],
                )
            return t

        def projection_T_dr(w8, srcT8, dst, psum_pool, drain):
            """dst [128, 4*1024] bf16 = descale * (W.T @ srcT), per d-tile.
            drain: 'dve' or 'act' or callable(t)->str."""
            wr = w8.rearrange("p (c n) -> p c n", c=NDC)
            sr = srcT8.rearrange("p (c l) -> p c l", c=NDC)
            for t in range(NDC):
                ps = psum_pool.tile([128, 1024], f32, tag="proj_ps")
                for lh in range(2):
                    for c2 in range(2):
                        nc.tensor.matmul(
                            ps[:, 512 * lh:512 * (lh + 1)],
                            wr[:, 2 * c2:2 * c2 + 2, 128 * t:128 * (t + 1)],
                            sr[:, 2 * c2:2 * c2 + 2, 512 * lh:512 * (lh + 1)],
                            start=(c2 == 0), stop=(c2 == 1),
                            perf_mode=DR,
                        )
                dsl = dst[:, 1024 * t:1024 * (t + 1)]
                eng = drain(t) if callable(drain) else drain
                if eng == "dve":
                    nc.vector.tensor_scalar_mul(out=dsl, in0=ps, scalar1=DSC)
                else:
                    nc.scalar.activation(out=dsl, in_=ps, func=AF.Identity,
                                         scale=DSC)

        def projection_nat_v_dr(w8, srcT8, v_bf, psum_pool, drain="dve"):
            """v_bf [128, 8*520]: natural V per l-tile (descaled); ones col."""
            wr = w8.rearrange("p (c n) -> p c n", c=NDC)
            sr = srcT8.rearrange("p (c l) -> p c l", c=NDC)
            for lt in range(NLT):
                ps = psum_pool.tile([128, 512], f32, tag="v_ps")
                for c2 in range(2):
                    nc.tensor.matmul(
                        ps,
                        sr[:, 2 * c2:2 * c2 + 2, 128 * lt:128 * (lt + 1)],
                        wr[:, 2 * c2:2 * c2 + 2, :],
                        start=(c2 == 0), stop=(c2 == 1),
                        perf_mode=DR,
                    )
                dst = v_bf[:, 520 * lt:520 * (lt + 1)].rearrange(
                    "p (h k) -> p h k", k=65
                )[:, :, 0:64]
                src = ps.rearrange("p (h k) -> p h k", k=64)
                eng = drain(lt) if callable(drain) else drain
                if eng == "dve":
                    nc.vector.tensor_scalar_mul(out=dst, in0=src, scalar1=DSC)
                else:
                    nc.scalar.activation(out=dst, in_=src, func=AF.Identity,
                                         scale=DSC)

        def make_v_tile(tag):
            v_bf = bfbuf.tile([128, NLT * 520], bf16, tag=tag)
            nc.vector.memset(
                v_bf.rearrange("p (th k) -> p th k", k=65)[:, :, 64:65], 1.0
            )
            return v_bf

        def attention_loop(wo, kT, qT, v_bf, resid_in_sb, resid_out_sb,
                           tail_cb, half0_cb=None, kt2_cb=None):
            """Banded softmax attention + out-proj + residual.
            tail_cb(t, pool): moving average + LN stats for tile t.
            half0_cb(): emitted after tail(3) at kt==5 (LN half-0 finish).
            kt2_cb(): emitted at the top of the kt==2 iteration (used to
            inject this attention's own lh=1 q projection)."""
            # o_norm/oT are transient per q-tile: 2-slot rings (qt % 2)
            o_norm = bfbuf.tile([128, 2 * 512], bf16, tag="o_norm")
            oT = bfbuf.tile([128, NDC * 2 * 128], bf16, tag="oT")
            expts = {}

            with tc.tile_pool(name="score_ps_pool", space="PSUM", bufs=2) as sp, \
                 tc.tile_pool(name="av_ps_pool", space="PSUM", bufs=2) as avp, \
                 tc.tile_pool(name="movtail_ps", space="PSUM", bufs=2) as mtp:

                def scores_exp(kt):
                    q_lo = max(0, 128 * kt - 64)
                    q_hi = min(L, 128 * kt + 192)
                    c_lo = q_lo - (128 * kt - 64)
                    c_hi = q_hi - (128 * kt - 64)
                    et = expp.tile([128, H * WIN], fp8, tag="expT")
                    expts[kt] = et
                    d3 = d_cat8.rearrange("p (c n) -> p c n", c=2)
                    i3 = ident8.rearrange("p (c n) -> p c n", c=2)
                    for g in range(2):  # 4-head super-groups; 2 banks each
                        ps = sp.tile([128, 1024], f32, tag="score_ps")
                        for hh in range(4):
                            h = 4 * g + hh
                            po = 64 * (h % 2)
                            co = 1024 * (h // 2)
                            # bias preload; region matches the kq accumulate
                            nc.tensor.matmul(
                                ps[:, WIN * hh + c_lo:WIN * hh + c_hi],
                                i3, d3[:, :, WIN * hh + c_lo:WIN * hh + c_hi],
                                start=True, stop=False,
                                perf_mode=DR,
                            )
                            nc.tensor.matmul(
                                ps[:, WIN * hh + c_lo:WIN * hh + c_hi],
                                kT[po:po + 64, co + 128 * kt:co + 128 * (kt + 1)],
                                qT[po:po + 64, co + q_lo:co + q_hi],
                                start=False, stop=True,
                            )
                        nc.scalar.activation(
                            out=et[:, 4 * WIN * g:4 * WIN * (g + 1)].rearrange(
                                "p (h w) -> p h w", w=WIN)[:, :, c_lo:c_hi],
                            in_=ps.rearrange("p (h w) -> p h w", w=WIN)[:, :, c_lo:c_hi],
                            func=AF.Exp,
                            scale=EXP_SCALE,
                        )

                def av_block(qt):
                    for g in range(2):
                        # full-bank tile so the 260-col view never crosses
                        # a PSUM bank boundary
                        ops5 = avp.tile([128, 512], f32, tag="small_ps")
                        ops = ops5[:, 0:4 * 65]
                        for hh in range(4):
                            h = 4 * g + hh
                            o = 65 * hh
                            last = ("r" if qt + 1 < NLT else "l")
                            # diagonal k-tile: q-window cols [64, 192)
                            nc.tensor.matmul(
                                ops[:, o:o + 65],
                                expts[qt][:, WIN * h + 64:WIN * h + 192],
                                v_bf[:, 520 * qt + 65 * h:520 * qt + 65 * (h + 1)],
                                start=True, stop=False,
                                skip_group_check=True,
                            )
                            if qt >= 1:  # k-tile qt-1 covers q_local [0, 64)
                                nc.tensor.matmul(
                                    ops[0:64, o:o + 65],
                                    expts[qt - 1][:, WIN * h + 192:WIN * h + 256],
                                    v_bf[:, 520 * (qt - 1) + 65 * h:
                                         520 * (qt - 1) + 65 * (h + 1)],
                                    start=False, stop=(last == "l"),
                                    skip_group_check=True,
                                )
                            if qt + 1 < NLT:  # k-tile qt+1 covers [64, 128)
                                nc.tensor.matmul(
                                    ops[64:128, o:o + 65],
                                    expts[qt + 1][:, WIN * h:WIN * h + 64],
                                    v_bf[:, 520 * (qt + 1) + 65 * h:
                                         520 * (qt + 1) + 65 * (h + 1)],
                                    start=False, stop=(last == "r"),
                                    skip_group_check=True,
                                )
                        rec = small.tile([128, 4], f32, tag="rec")
                        nc.vector.reciprocal(
                            out=rec,
                            in_=ops.rearrange("p (h k) -> p h k", k=65)[:, :, 64:65],
                        )
                        sl = 512 * (qt % 2)
                        nc.vector.tensor_tensor(
                            out=o_norm[:, sl + 256 * g:
                                       sl + 256 * (g + 1)].rearrange(
                                "p (h d) -> p h d", d=64),
                            in0=ops.rearrange("p (h k) -> p h k", k=65)[:, :, 0:64],
                            in1=bcast64(rec),
                            op=ALU.mult,
                        )
                    sl = 512 * (qt % 2)
                    for j in range(NDC):
                        nc.sync.dma_start_transpose(
                            out=oT[:, 256 * j + 128 * (qt % 2):
                                   256 * j + 128 * (qt % 2) + 128],
                            in_=o_norm[:, sl + 128 * j:sl + 128 * (j + 1)],
                        )

                def out_proj(lt):
                    ps = avp.tile([128, 512], f32, tag="small_ps")
                    for c in range(NDC):
                        nc.tensor.matmul(
                            ps,
                            oT[:, 256 * c + 128 * (lt % 2):
                               256 * c + 128 * (lt % 2) + 128],
                            wo[:, 512 * c:512 * (c + 1)],
                            start=(c == 0), stop=(c == NDC - 1),
                        )
                    nc.vector.tensor_tensor(
                        out=resid_out_sb[:, 512 * lt:512 * (lt + 1)],
                        in0=ps,
                        in1=resid_in_sb[:, 512 * lt:512 * (lt + 1)],
                        op=ALU.add,
                    )

                for kt in range(NLT):
                    if kt == 2 and kt2_cb is not None:
                        kt2_cb(mtp)
                    scores_exp(kt)
                    if kt >= 1:
                        av_block(kt - 1)
                        out_proj(kt - 1)
                    if kt >= 2:
                        expts.pop(kt - 3, None)
                        tail_cb(kt - 2, mtp)
                    if kt == 5 and half0_cb is not None:
                        half0_cb()
                av_block(NLT - 1)
                out_proj(NLT - 1)
                tail_cb(NLT - 2, mtp)
                tail_cb(NLT - 1, mtp)

        def make_mov_tail(in_sb, mv, psum_tag="mov_ps"):
            """Returns (cb, mov_sb): cb(t, pool) emits the banded A @ in_sb
            matmuls, the bf16 drain, and LN stats for tile t."""
            ensure_a()
            mov_sb = movp.tile([128, NLT * 512], bf16, tag="mov")

            def cb(t, pool):
                ps = pool.tile([128, 512], f32, tag=psum_tag)
                js = [j for j in (t - 1, t, t + 1) if 0 <= j < NLT]
                for ji, j in enumerate(js):
                    bi = a_blocks[(t, j)]
                    nc.tensor.matmul(
                        ps,
                        a_sb[:, 128 * bi:128 * (bi + 1)],
                        in_sb[:, 512 * j:512 * (j + 1)],
                        start=(ji == 0), stop=(ji == len(js) - 1),
                    )
                if t % 2 == 0 or t >= 6:
                    nc.scalar.copy(out=mov_sb[:, 512 * t:512 * (t + 1)],
                                   in_=ps)
                else:
                    nc.vector.tensor_copy(out=mov_sb[:, 512 * t:512 * (t + 1)],
                                          in_=ps)
                st6 = small.tile([128, 6], f32, tag="st6")
                if t >= 6:  # last tiles: stats from PSUM, parallel with drain
                    nc.vector.bn_stats(out=st6, in_=ps)
                else:
                    nc.vector.bn_stats(out=st6,
                                       in_=mov_sb[:, 512 * t:512 * (t + 1)])
                nc.vector.bn_aggr(out=mv[:, 2 * t:2 * (t + 1)], in_=st6)

            return cb, mov_sb

        def make_ln_half(mov_sb, mv, n_bf=None, nT=None, nT8=None, q8scale=SX,
                         n_sb=None, out_dma=None, bf_scale=1.0, eps=EPS):
            """Returns (main_cb(half), quant_cb(half)). main: rstd via Act
            Sqrt + DVE reciprocal, then per-tile normalize (DVE 4x) +
            SBUF->SBUF transposes / output DMA. quant: one Act op per half
            (bf16 -> fp8), emitted separately so the caller controls the
            Act queue order."""
            mv3 = mv.rearrange("p (t two) -> p t two", two=2)
            rstds = {}

            def main_cb(half, nbf_eng=None, t0=None, nt=4):
                t0 = 4 * half if t0 is None else t0
                # rstd = 1/sqrt(var+eps): quake seed + 1 Newton step, all on
                # DVE (an Act Sqrt would thrash the activation table between
                # the surrounding Exp/Gelu ops, costing ~1.3us per reload)
                vv_t = stats_p.tile([128, 4], f32, tag="vv")
                vv = vv_t[:, :nt]
                nc.vector.tensor_scalar_add(out=vv, in0=mv3[:, t0:t0 + nt, 1:2],
                                            scalar1=eps)
                rstd_t = stats_p.tile([128, 4], f32, tag="rstd")
                rstd = rstd_t[:, :nt]
                yi = rstd.bitcast(mybir.dt.int32)
                nc.vector.tensor_scalar(
                    out=yi, in0=vv.bitcast(mybir.dt.int32),
                    scalar1=1, scalar2=None, op0=ALU.arith_shift_right,
                )
                nc.vector.tensor_scalar(
                    out=yi, in0=yi, scalar1=-1, scalar2=0x5F3759DF,
                    op0=ALU.mult, op1=ALU.add,
                )
                t1_t = stats_p.tile([128, 4], f32, tag="t1")
                t1 = t1_t[:, :nt]
                nc.vector.tensor_tensor(out=t1, in0=rstd, in1=rstd, op=ALU.mult)
                nc.vector.tensor_tensor(out=t1, in0=t1, in1=vv, op=ALU.mult)
                nc.vector.tensor_scalar(
                    out=t1, in0=t1, scalar1=-0.5, scalar2=1.5,
                    op0=ALU.mult, op1=ALU.add,
                )
                nc.vector.tensor_tensor(out=rstd, in0=rstd, in1=t1,
                                        op=ALU.mult)
                rstds[half] = rstd
                rstd_bf = rstd
                if bf_scale != 1.0:
                    rstd_bf_t = stats_p.tile([128, 4], f32, tag="rstd_bf")
                    rstd_bf = rstd_bf_t[:, :nt]
                    nc.vector.tensor_scalar_mul(out=rstd_bf, in0=rstd,
                                                scalar1=bf_scale)
                for tt in range(nt):
                    t = t0 + tt
                    dst = n_bf if n_bf is not None else n_sb
                    (nbf_eng or nc.vector).tensor_scalar(
                        out=dst[:, 512 * t:512 * (t + 1)],
                        in0=mov_sb[:, 512 * t:512 * (t + 1)],
                        scalar1=mv[:, 2 * t:2 * t + 1],
                        scalar2=rstd_bf[:, tt:tt + 1],
                        op0=ALU.subtract,
                        op1=ALU.mult,
                    )
                    if nT is not None:
                        for j in range(NDC):
                            nc.sync.dma_start_transpose(
                                out=nT[:, 1024 * j + 128 * t:
                                       1024 * j + 128 * (t + 1)],
                                in_=n_bf[:, 512 * t + 128 * j:
                                         512 * t + 128 * (j + 1)],
                            )
                    if out_dma is not None:
                        deng = nc.sync if t % 2 == 0 else nc.scalar
                        deng.dma_start(
                            out=out_dma[128 * t:128 * (t + 1), :],
                            in_=n_sb[:, 512 * t:512 * (t + 1)],
                        )

            def quant_cb(half, split=False, pool=False):
                nr = nT.rearrange("p (c l) -> p c l", c=NDC)
                n8r = nT8.rearrange("p (c l) -> p c l", c=NDC)
                sl = slice(512 * half, 512 * (half + 1))
                if pool:  # idle GpSimd: off the loop-saturated DVE/Act
                    nc.gpsimd.tensor_scalar_mul(
                        out=n8r[:, :, sl], in0=nr[:, :, sl], scalar1=q8scale,
                    )
                elif split:  # halve latency: chunks 0-1 on DVE, 2-3 on Act
                    nc.vector.tensor_scalar_mul(
                        out=n8r[:, 0:2, sl], in0=nr[:, 0:2, sl],
                        scalar1=q8scale,
                    )
                    nc.scalar.activation(
                        out=n8r[:, 2:4, sl], in_=nr[:, 2:4, sl],
                        func=AF.Identity, scale=q8scale,
                    )
                else:
                    nc.scalar.activation(
                        out=n8r[:, :, sl], in_=nr[:, :, sl],
                        func=AF.Identity, scale=q8scale,
                    )

            return main_cb, quant_cb

        def projection_q8_dr(w8, srcT8, qT, lh, pool=None):
            """Q projection (fp8 src) for one l-half; drains split DVE/Act."""
            wr = w8.rearrange("p (c n) -> p c n", c=NDC)
            sr = srcT8.rearrange("p (c l) -> p c l", c=NDC)

            def emit(qp, tag):
                for t in range(NDC):
                    ps = qp.tile([128, 512], f32, tag=tag)
                    for c2 in range(2):
                        nc.tensor.matmul(
                            ps,
                            wr[:, 2 * c2:2 * c2 + 2, 128 * t:128 * (t + 1)],
                            sr[:, 2 * c2:2 * c2 + 2, 512 * lh:512 * (lh + 1)],
                            start=(c2 == 0), stop=(c2 == 1),
                            perf_mode=DR,
                        )
                    dsl = qT[:, 1024 * t + 512 * lh:1024 * t + 512 * (lh + 1)]
                    if t % 2 == 0:
                        nc.vector.tensor_scalar_mul(out=dsl, in0=ps,
                                                    scalar1=DSC)
                    else:
                        nc.scalar.activation(out=dsl, in_=ps, func=AF.Identity,
                                             scale=DSC)

            if pool is not None:
                emit(pool, "mov_ps")
            else:
                with tc.tile_pool(name="q_ps_pool", space="PSUM", bufs=2) as qp:
                    emit(qp, "proj_ps")

        # ================= the layer =================
        for _rep in range(reps):
            # DMA: SP queue carries xT8/x_sb + tiny constants (then stays
            # free for the in-loop transposes); the gpsimd SWDGE queue
            # carries all weights, ordered by first use.
            xT8 = load_srcT8(xT8_d, "srcT8")
            with tc.tile_pool(name="attn_w", bufs=1) as wpool:
                wk1 = load_w8(wpool, wk_sa8, "wk8", eng=nc.gpsimd)
                wq1 = load_w8(wpool, wq_sa8, "wq8", eng=nc.gpsimd)
                wv1 = load_w8(wpool, wv_sa8, "wv8", eng=nc.gpsimd)
                x_sb = bfbuf.tile([128, NLT * 512], bf16, tag="x_sb")
                nc.sync.dma_start(
                    out=x_sb.rearrange("p (t d) -> p t d", t=NLT),
                    in_=x_bf_d.rearrange("(t p) d -> p t d", p=128),
                )
                encT8 = load_srcT8(encT8_d, "srcT8", eng=nc.gpsimd)
                wk2 = load_w8(wpool, wk_ca8, "wk8_ca", eng=nc.gpsimd)
                wv2 = load_w8(wpool, wv_ca8, "wv8_ca", eng=nc.gpsimd)
                wo1 = load_w16(wpool, wo_sa, "wo", eng=nc.gpsimd)
                ensure_a()
                wq2 = load_w8(wpool, wq_ca8, "wq8_ca", eng=nc.gpsimd)
                wo2 = load_w16(wpool, wo_ca, "wo_ca", eng=nc.gpsimd)
                w1 = wpool.tile([128, NDC * DFF], fp8, tag="w1")
                nc.gpsimd.dma_start(
                    out=w1.rearrange("p (c n) -> p c n", c=NDC),
                    in_=w18.rearrange("(c p) n -> p c n", p=128),
                )
                w2 = wpool.tile([128, NFT * 512], fp8, tag="w2")
                nc.gpsimd.dma_start(
                    out=w2.rearrange("p (c n) -> p c n", c=NFT),
                    in_=w28.rearrange("(c p) n -> p c n", p=128),
                )

                # --- ALL k/v projections (SA + CA) share one PSUM scope so
                # the SA->CA boundary only has the q2 projections left ---
                kT1 = bfbuf.tile([128, NDC * 1024], bf16, tag="kT")
                qT1 = bfbuf.tile([128, NDC * 1024], bf16, tag="qT")
                kT2 = bfbuf.tile([128, NDC * 1024], bf16, tag="kT2")
                v1 = make_v_tile("v_bf")
                v2 = make_v_tile("v_bf2")
                qT2 = bfbuf.tile([128, NDC * 1024], bf16, tag="qT")
                with tc.tile_pool(name="kv_ps", space="PSUM", bufs=3) as kvp, \
                     tc.tile_pool(name="kv_v", space="PSUM", bufs=2) as kvv:
                    projection_T_dr(wk1, xT8, kT1, kvp, "dve")
                    projection_T_dr(wq1, xT8, qT1, kvp, "act")
                    projection_nat_v_dr(wv1, xT8, v1, kvv, "dve")
                    projection_T_dr(wk2, encT8, kT2, kvp,
                                    lambda t: "dve" if t < 2 else "act")
                    projection_nat_v_dr(wv2, encT8, v2, kvv,
                                        lambda lt: "dve" if lt % 2 else "act")

                # --- self attention + residual (mov1+stats in the loop) ---
                r1 = streams.tile([128, NLT * 512], f32r, tag="stream")
                mv1 = stats_p.tile([128, NLT * 2], f32, tag="mv")
                mov1_cb, mov1 = make_mov_tail(r1, mv1)
                n1_bf = bfbuf.tile([128, NLT * 512], bf16, tag="n_bf")
                n1T = srcp16.tile([128, NDC * 1024], bf16, tag="srcT")
                n1T8 = bfbuf.tile([128, NDC * 1024], fp8, tag="n1T8")
                ln1_main, ln1_quant = make_ln_half(mov1, mv1, n_bf=n1_bf,
                                                   nT=n1T, nT8=n1T8)

                def ln1_h0():
                    ln1_main(0, nbf_eng=nc.gpsimd)
                    ln1_quant(0, pool=True)

                attention_loop(wo1, kT1, qT1, v1, x_sb, r1,
                               tail_cb=mov1_cb, half0_cb=ln1_h0)
                ln1_main(1)

                # --- CA q projection lh=0 only; lh=1 (and its h1 quant) are
                # injected into the CA loop at kt==2 — CA scores for kt<=2
                # read q columns < 512, so the loop starts immediately ---
                projection_q8_dr(wq2, n1T8, qT2, 0)

                def ca_kt2(pool):
                    ln1_quant(1, split=True)
                    projection_q8_dr(wq2, n1T8, qT2, 1, pool=pool)

                # --- cross attention + residual (mov2+stats in the loop) ---
                r2 = streams.tile([128, NLT * 512], f32r, tag="stream")
                mv2 = stats_p.tile([128, NLT * 2], f32, tag="mv")
                mov2_cb, mov2 = make_mov_tail(r2, mv2)
                n2_bf = bfbuf.tile([128, NLT * 512], bf16, tag="n_bf")
                n2T = srcp16.tile([128, NDC * 1024], bf16, tag="srcT")
                # shares n1T8's buffer: n1T8's last read (CA q lh=1) is long
                # before the first n2T8 write (CA kt==5 quant)
                n2T8 = bfbuf.tile([128, NDC * 1024], fp8, tag="n1T8")
                # n2_bf carries x SW2 (the FFN residual base); n2T8 is x SX.
                ln2_main, ln2_quant = make_ln_half(mov2, mv2, n_bf=n2_bf,
                                                   nT=n2T, nT8=n2T8,
                                                   q8scale=SX / SW2,
                                                   bf_scale=SW2)

                def ln2_h0():
                    ln2_main(0, nbf_eng=nc.gpsimd)
                    ln2_quant(0, pool=True)

                attention_loop(wo2, kT2, qT2, v2, n1_bf, r2,
                               tail_cb=mov2_cb, half0_cb=ln2_h0,
                               kt2_cb=ca_kt2)
                ln2_main(1)

                # --- FFN (fp8 DoubleRow; r3 carries x SW2, LN3 absorbs it) ---
                r3 = streams.tile([128, NLT * 512], f32r, tag="stream")
                mv3 = stats_p.tile([128, NLT * 2], f32, tag="mv")
                mov3_cb, mov3 = make_mov_tail(r3, mv3, psum_tag="ff2_ps")
                out_sb = streams.tile([128, NLT * 512], bf16, tag="stream")
                ln3_main, _ = make_ln_half(mov3, mv3, n_sb=out_sb,
                                           out_dma=out_d, eps=EPS * SW2 * SW2)
                with tc.tile_pool(name="h_psp", space="PSUM", bufs=2) as hps, \
                     tc.tile_pool(name="ff2_psp", space="PSUM", bufs=3) as f2ps:
                    w1r = w1.rearrange("p (c n) -> p c n", c=NDC)
                    w2r = w2.rearrange("p (c n) -> p c n", c=NFT)
                    n2r = n2T8.rearrange("p (c l) -> p c l", c=NDC)
                    g1T_a = bfbuf.tile([128, NFT * 512], fp8, tag="g1T")
                    g1T_b = bfbuf.tile([128, NFT * 512], fp8, tag="g1T_b")
                    g1Ts = [g1T_a, g1T_b]

                    def ffn1_f2(lh, f2):
                        ps = hps.tile([128, 1024], f32, tag="h_ps")
                        for fh in range(2):
                            f = 2 * f2 + fh
                            for c2 in range(2):
                                nc.tensor.matmul(
                                    ps[:, 512 * fh:512 * (fh + 1)],
                                    w1r[:, 2 * c2:2 * c2 + 2,
                                        128 * f:128 * (f + 1)],
                                    n2r[:, 2 * c2:2 * c2 + 2,
                                        512 * lh:512 * (lh + 1)],
                                    start=(c2 == 0), stop=(c2 == 1),
                                    perf_mode=DR,
                                )
                        nc.scalar.activation(
                            out=g1Ts[lh][:, 1024 * f2:1024 * (f2 + 1)],
                            in_=ps, func=AF.Gelu, scale=1.0 / (SX * SW),
                        )

                    def ffn2_tile(lh, ltt):
                        lt = 4 * lh + ltt
                        g1r = g1Ts[lh].rearrange("p (c n) -> p c n", c=NFT)
                        ps = f2ps.tile([128, 512], f32, tag="ff2_ps")
                        for c2 in range(NFT // 2):
                            nc.tensor.matmul(
                                ps,
                                g1r[:, 2 * c2:2 * c2 + 2,
                                    128 * ltt:128 * (ltt + 1)],
                                w2r[:, 2 * c2:2 * c2 + 2, :],
                                start=(c2 == 0), stop=(c2 == NFT // 2 - 1),
                                perf_mode=DR,
                            )
                        nc.vector.tensor_tensor(
                            out=r3[:, 512 * lt:512 * (lt + 1)],
                            in0=ps,
                            in1=n2_bf[:, 512 * lt:512 * (lt + 1)],
                            op=ALU.add,
                        )

                    # lh0 FFN1 (h0's LN chain starts at kt==5 and its
                    # quant runs on Pool, so it is ready first)
                    for f2 in range(NFT // 2):
                        ffn1_f2(0, f2)
                    # lh1 FFN1 interleaved with lh0 FFN2 (fills PE idle)
                    ln2_quant(1, split=True)
                    for f2 in range(NFT // 2):
                        ffn1_f2(1, f2)
                        if f2 % 2 == 1:
                            ltt = f2 // 2
                            ffn2_tile(0, ltt)
                            if ltt >= 1:
                                mov3_cb(ltt - 1, f2ps)
                            if ltt == 2:
                                ln3_main(0, t0=0, nt=2)
                    for ltt in range(4):
                        lt = 4 + ltt
                        ffn2_tile(1, ltt)
                        mov3_cb(lt - 1, f2ps)
                        if lt == 5:
                            ln3_main(1, t0=2, nt=2)
                        if lt == 7:
                            ln3_main(2, t0=4, nt=2)
                    mov3_cb(NLT - 1, f2ps)
                    ln3_main(3, t0=6, nt=2)

    nc.compile()
    _CACHE[key] = nc
    return nc


def _make_in_maps(inputs):
    d_cat8, a_strip = _host_constants()

    def T(w):
        return np.ascontiguousarray(np.asarray(w, dtype=np.float32).T)

    def T8(w, s):
        return (T(w) * s).astype(F8)

    ident8 = np.concatenate([np.eye(128, dtype=np.float32)] * 2, axis=1)
    common = {
        "wq_sa8": T8(inputs["sa_Wq"], SW),
        "wk_sa8": T8(inputs["sa_Wk"], SW),
        "wv_sa8": T8(inputs["sa_Wv"], SW),
        "wo_sa": T(inputs["sa_Wo"]).astype(BF16),
        "wq_ca8": T8(inputs["ca_Wq"], SW),
        "wk_ca8": T8(inputs["ca_Wk"], SW),
        "wv_ca8": T8(inputs["ca_Wv"], SW),
        "wo_ca": T(inputs["ca_Wo"]).astype(BF16),
        "w18": T8(inputs["ff_W1"], SW),
        "w28": T8(inputs["ff_W2"], SW2),
        "d_cat8": d_cat8,
        "a_strip": a_strip,
        "ident8": ident8.astype(F8),
    }
    x = np.asarray(inputs["x"], dtype=np.float32)
    enc = np.asarray(inputs["enc_out"], dtype=np.float32)
    maps = []
    for b in range(B):
        m = dict(common)
        m["xT8"] = np.ascontiguousarray(x[b].T * SX).astype(F8)
        m["encT8"] = np.ascontiguousarray(enc[b].T * SX).astype(F8)
        m["x_bf"] = np.ascontiguousarray(x[b]).astype(BF16)
        maps.append(m)
    return maps


def kernel(**inputs):
    from concourse.bass_utils import run_bass_kernel_spmd

    nc = _build_program()
    in_maps = _make_in_maps(inputs)
    res = run_bass_kernel_spmd(nc, in_maps, list(range(B)))
    _CACHE["last_results"] = res
    out = np.stack([np.asarray(res.results[b]["out"]) for b in range(B)])
    return out.astype(np.float32)


# revision 59
# speedup vs baseline: 1.1178x; 1.1178x over previous
# Trainium2 Bass kernel for nn_AutoformerDecoderLayer (B=8,L=1024,D=512,DFF=2048,H=8,DK=64)
# Strategy: data-parallel over batch B across 8 NeuronCores (zero collectives).
# Each core runs the full decoder layer on one [1024, 512] batch element.
#
# v5 design notes (on top of v3; sim span 170us -> 135us):
#  - q/k drains descale by 1/(SX*SW) so exp scale is 1/8 (natural units);
#    the ALiBi bias is preloaded as 8*bias split hi+lo into two fp8 chunks
#    via one DoubleRow matmul per head window (half the PE cost of the v3
#    bf16 ident preloads). Scores run as 2 super-groups of 4 heads per
#    k-tile ([128,1024] PSUM tiles) so exp is 2 Act ops per k-tile, not 4.
#  - exp output (softmax weights) is fp8; the AV matmul mixes fp8
#    stationary with bf16 moving V.
#  - Bulk weight/input DMA rides the gpsimd SWDGE queue; the SP HWDGE
#    queue stays free for the latency-critical SBUF->SBUF transposes
#    (oT and the LN nT transposes - no DRAM round trip).
#  - ALL k/v projections (SA + CA) run in one PSUM scope in the prologue
#    (k/q/v drains split across DVE and Act), so the SA->CA boundary only
#    has the q2 projections; q2 lh=1 is injected into the CA loop at kt==2
#    (scores kt<=2 read q cols < 512).
#  - mov drains write bf16 (Act/DVE split; last tiles on Act); LN stats
#    for the last tiles read PSUM directly. LN normalize runs as DVE
#    tensor_scalar in 4x mode; LN half-0 normalize + quant run on the
#    otherwise-idle GpSimd(Pool) engine inside the attention loops.
#  - rstd = quake rsqrt + 1 Newton step, DVE only (an Act Sqrt would
#    thrash the activation table against Exp/Gelu: ~1.3us per reload).
#  - LN half 0 is emitted inside the attention loop at kt==5 (stats for
#    tiles 0-3 ready); FFN1 lh=1 is interleaved with FFN2 lh=0; LN3 is
#    emitted in pairs as mov3 stats complete, with output DMAs alternating
#    the SP/Act queues. Output is bf16 (converted to f32 on host).
import sys

sys.path.insert(0, "/opt/trn_rl_repo")

from contextlib import ExitStack

import numpy as np
import ml_dtypes

B, L, D, DFF, H, DK = 8, 1024, 512, 2048, 8, 64
KSZ = 25
PAD = KSZ // 2
EPS = 1e-5
NLT = L // 128      # 8 l-tiles
NDC = D // 128      # 4 d-chunks
NFT = DFF // 128    # 16 dff tiles
BF16 = ml_dtypes.bfloat16
F8 = ml_dtypes.float8_e4m3

SW = 256.0    # fp8 weight scale (qkv / W1)
SX = 16.0     # fp8 activation scale
SW2 = 64.0    # FFN2 weight scale == r3 residual scale (LN3 absorbs it)
DSC = 1.0 / (SX * SW)   # q/k drain descale -> natural units
EXP_SCALE = 1.0 / 8.0   # 1/sqrt(DK)
WIN = 256     # per-k-tile q window; starts at 128*kt - 64
_CACHE = {}


def _host_constants():
    # Bias for the win-256 window: k = 128*kt + i, q = 128*kt-64 + c.
    # Preloaded into PSUM as 8*bias = hi + lo (two fp8 chunks), duplicated
    # across the 4 heads of a super-group: d_cat8 [128, 2*4*WIN].
    i = np.arange(128)[:, None].astype(np.float64)
    c = np.arange(WIN)[None, :].astype(np.float64)
    b8 = 8.0 * (-0.1 * np.abs(c - 64.0 - i))          # [128, 256] in [-154, 0]
    hi = b8.astype(F8)
    lo = (b8 - hi.astype(np.float64)).astype(F8)
    d_cat8 = np.concatenate([np.tile(hi, (1, 4)), np.tile(lo, (1, 4))], axis=1)

    # Moving-average matrix A[lo, li] = 1/25 iff |lo-li| <= 12, packed into
    # the exact a_sb SBUF layout: 22 banded [128, 128] blocks side by side.
    lo_i = np.arange(L)[:, None]
    li = np.arange(L)[None, :]
    A = ((np.abs(lo_i - li) <= PAD).astype(np.float64) / KSZ).astype(np.float32)
    blocks = []
    for t in range(NLT):
        for j in range(max(0, t - 1), min(NLT, t + 2)):
            blocks.append(A[128 * j:128 * (j + 1), 128 * t:128 * (t + 1)])
    a_strip = np.concatenate(blocks, axis=1)  # [128, 22*128]
    return d_cat8, a_strip


def _build_program(reps=1):
    """Build (and cache) the single-core Bass program + compile it.

    reps>1 repeats the whole layer body (timing calibration only)."""
    key = ("nc", reps)
    if key in _CACHE:
        return _CACHE[key]

    import concourse.tile as tile
    import concourse.mybir as mybir
    from concourse import bacc
    from concourse.bass import AP as BassAP

    f32 = mybir.dt.float32
    f32r = mybir.dt.float32r
    bf16 = mybir.dt.bfloat16
    fp8 = mybir.dt.float8e4
    AF = mybir.ActivationFunctionType
    ALU = mybir.AluOpType
    DR = mybir.MatmulPerfMode.DoubleRow

    nc = bacc.Bacc("TRN2", target_bir_lowering=False, debug=False)

    # ---------------- DRAM parameters (per-core shapes) ----------------
    def din(name, shape, dt=f32):
        return nc.dram_tensor(name, list(shape), dt, kind="ExternalInput").ap()

    xT8_d = din("xT8", (D, L), fp8)      # x.T * SX
    encT8_d = din("encT8", (D, L), fp8)  # enc.T * SX
    x_bf_d = din("x_bf", (L, D), bf16)   # residual base
    wq_sa8 = din("wq_sa8", (D, D), fp8)  # W.T * SW
    wk_sa8 = din("wk_sa8", (D, D), fp8)
    wv_sa8 = din("wv_sa8", (D, D), fp8)
    wo_sa = din("wo_sa", (D, D), bf16)
    wq_ca8 = din("wq_ca8", (D, D), fp8)  # W.T * SW
    wk_ca8 = din("wk_ca8", (D, D), fp8)
    wv_ca8 = din("wv_ca8", (D, D), fp8)
    wo_ca = din("wo_ca", (D, D), bf16)
    w18 = din("w18", (D, DFF), fp8)      # W1.T * SW
    w28 = din("w28", (DFF, D), fp8)      # W2.T * SW2
    d_cat8_d = din("d_cat8", (128, 2 * 4 * WIN), fp8)
    a_strip_d = din("a_strip", (128, 22 * 128), f32r)
    ident8_d = din("ident8", (128, 2 * 128), fp8)
    out_d = nc.dram_tensor("out", [L, D], bf16, kind="ExternalOutput").ap()

    with tile.TileContext(nc) as tc, ExitStack() as ctx:
        persist = ctx.enter_context(tc.tile_pool(name="persist", bufs=1))
        streams = ctx.enter_context(tc.tile_pool(name="streams", bufs=2))
        movp = ctx.enter_context(tc.tile_pool(name="movp", bufs=1))
        srcp8 = ctx.enter_context(tc.tile_pool(name="srcp8", bufs=2))
        srcp16 = ctx.enter_context(tc.tile_pool(name="srcp16", bufs=1))
        bfbuf = ctx.enter_context(tc.tile_pool(name="bfbuf", bufs=1))
        nbf_p = ctx.enter_context(tc.tile_pool(name="nbf_p", bufs=3))
        expp = ctx.enter_context(tc.tile_pool(name="expp", bufs=4))
        stats_p = ctx.enter_context(tc.tile_pool(name="stats", bufs=2))
        small = ctx.enter_context(tc.tile_pool(name="small", bufs=4))

        # ---------- tiny constants ----------
        d_cat8 = persist.tile([128, 2 * 4 * WIN], fp8, tag="d_cat8")
        nc.sync.dma_start(out=d_cat8, in_=d_cat8_d)
        ident8 = persist.tile([128, 2 * 128], fp8, tag="ident8")
        nc.sync.dma_start(out=ident8, in_=ident8_d)
        eps_sb = persist.tile([128, 1], f32, tag="eps")
        nc.vector.memset(eps_sb, EPS)
        eps3_sb = persist.tile([128, 1], f32, tag="eps3")
        nc.vector.memset(eps3_sb, EPS * SW2 * SW2)
        warm = persist.tile([128, 1], f32, tag="warm")
        nc.scalar.activation(out=warm, in_=eps_sb, func=AF.Exp)
        nc.scalar.activation(out=warm, in_=eps_sb, func=AF.Sqrt)
        nc.scalar.activation(out=warm, in_=eps_sb, func=AF.Gelu)

        a_sb = persist.tile([128, 22 * 128], f32r, tag="a_sb")
        a_blocks = {}
        bi = 0
        for t in range(NLT):
            for j in range(max(0, t - 1), min(NLT, t + 2)):
                a_blocks[(t, j)] = bi
                bi += 1
        a_loaded = [False]

        def ensure_a():
            if not a_loaded[0]:
                a_loaded[0] = True
                # two DMAs so the first tails' blocks land earlier
                nc.gpsimd.dma_start(out=a_sb[:, :11 * 128],
                                    in_=a_strip_d[:, :11 * 128])
                nc.gpsimd.dma_start(out=a_sb[:, 11 * 128:],
                                    in_=a_strip_d[:, 11 * 128:])

        def bcast64(ap):
            """[128, n] AP -> [128, n, 64] stride-0 broadcast AP."""
            return BassAP(ap.tensor, ap.offset, list(ap.ap) + [[0, 64]])

        # ================= helpers =================
        def load_w8(wpool, dram_ap, tag, eng=None):
            t = wpool.tile([128, NDC * 512], fp8, tag=tag)
            (eng or nc.sync).dma_start(
                out=t.rearrange("p (c n) -> p c n", c=NDC),
                in_=dram_ap.rearrange("(c p) n -> p c n", p=128),
            )
            return t

        def load_w16(wpool, dram_ap, tag, eng=None):
            t = wpool.tile([128, NDC * 512], bf16, tag=tag)
            (eng or nc.sync).dma_start(
                out=t.rearrange("p (c n) -> p c n", c=NDC),
                in_=dram_ap.rearrange("(c p) n -> p c n", p=128),
            )
            return t

        def load_srcT8(dram_ap, tag, eng=None):
            t = srcp8.tile([128, NDC * 1024], fp8, tag=tag)
            for lh in range(2):  # l-halves so the first projections start early
                (eng or nc.sync).dma_start(
                    out=t.rearrange("p (c l) -> p c l", c=NDC)[
                        :, :, 512 * lh:512 * (lh + 1)],
                    in_=dram_ap.rearrange("(c p) l -> p c l", p=128)[
                        :, :, 512 * lh:512 * (lh + 1)],
                )
            return t

        def projection_T_dr(w8, srcT8, dst, psum_pool, drain):
            """dst [128, 4*1024] bf16 = descale * (W.T @ srcT), per d-tile.
            drain: 'dve' or 'act' or callable(t)->str."""
            wr = w8.rearrange("p (c n) -> p c n", c=NDC)
            sr = srcT8.rearrange("p (c l) -> p c l", c=NDC)
            for t in range(NDC):
                ps = psum_pool.tile([128, 1024], f32, tag="proj_ps")
                for lh in range(2):
                    for c2 in range(2):
                        nc.tensor.matmul(
                            ps[:, 512 * lh:512 * (lh + 1)],
                            wr[:, 2 * c2:2 * c2 + 2, 128 * t:128 * (t + 1)],
                            sr[:, 2 * c2:2 * c2 + 2, 512 * lh:512 * (lh + 1)],
                            start=(c2 == 0), stop=(c2 == 1),
                            perf_mode=DR,
                        )
                dsl = dst[:, 1024 * t:1024 * (t + 1)]
                eng = drain(t) if callable(drain) else drain
                if eng == "dve":
                    nc.vector.tensor_scalar_mul(out=dsl, in0=ps, scalar1=DSC)
                else:
                    nc.scalar.activation(out=dsl, in_=ps, func=AF.Identity,
                                         scale=DSC)

        def projection_nat_v_dr(w8, srcT8, v_bf, psum_pool, drain="dve"):
            """v_bf [128, 8*520]: natural V per l-tile (descaled); ones col."""
            wr = w8.rearrange("p (c n) -> p c n", c=NDC)
            sr = srcT8.rearrange("p (c l) -> p c l", c=NDC)
            for lt in range(NLT):
                ps = psum_pool.tile([128, 512], f32, tag="v_ps")
                for c2 in range(2):
                    nc.tensor.matmul(
                        ps,
                        sr[:, 2 * c2:2 * c2 + 2, 128 * lt:128 * (lt + 1)],
                        wr[:, 2 * c2:2 * c2 + 2, :],
                        start=(c2 == 0), stop=(c2 == 1),
                        perf_mode=DR,
                    )
                dst = v_bf[:, 520 * lt:520 * (lt + 1)].rearrange(
                    "p (h k) -> p h k", k=65
                )[:, :, 0:64]
                src = ps.rearrange("p (h k) -> p h k", k=64)
                eng = drain(lt) if callable(drain) else drain
                if eng == "dve":
                    nc.vector.tensor_scalar_mul(out=dst, in0=src, scalar1=DSC)
                else:
                    nc.scalar.activation(out=dst, in_=src, func=AF.Identity,
                                         scale=DSC)

        def make_v_tile(tag):
            v_bf = bfbuf.tile([128, NLT * 520], bf16, tag=tag)
            nc.vector.memset(
                v_bf.rearrange("p (th k) -> p th k", k=65)[:, :, 64:65], 1.0
            )
            return v_bf

        def attention_loop(wo, kT, qT, v_bf, resid_in_sb, resid_out_sb,
                           tail_cb, half0_cb=None, kt2_cb=None):
            """Banded softmax attention + out-proj + residual.
            tail_cb(t, pool): moving average + LN stats for tile t.
            half0_cb(): emitted after tail(3) at kt==5 (LN half-0 finish).
            kt2_cb(): emitted at the top of the kt==2 iteration (used to
            inject this attention's own lh=1 q projection)."""
            # o_norm/oT are transient per q-tile: 2-slot rings (qt % 2)
            o_norm = bfbuf.tile([128, 2 * 512], bf16, tag="o_norm")
            oT = bfbuf.tile([128, NDC * 2 * 128], bf16, tag="oT")
            expts = {}

            with tc.tile_pool(name="score_ps_pool", space="PSUM", bufs=2) as sp, \
                 tc.tile_pool(name="av_ps_pool", space="PSUM", bufs=2) as avp, \
                 tc.tile_pool(name="movtail_ps", space="PSUM", bufs=2) as mtp:

                def scores_exp(kt):
                    q_lo = max(0, 128 * kt - 64)
                    q_hi = min(L, 128 * kt + 192)
                    c_lo = q_lo - (128 * kt - 64)
                    c_hi = q_hi - (128 * kt - 64)
                    et = expp.tile([128, H * WIN], fp8, tag="expT")
                    expts[kt] = et
                    d3 = d_cat8.rearrange("p (c n) -> p c n", c=2)
                    i3 = ident8.rearrange("p (c n) -> p c n", c=2)
                    for g in range(2):  # 4-head super-groups; 2 banks each
                        ps = sp.tile([128, 1024], f32, tag="score_ps")
                        for hh in range(4):
                            h = 4 * g + hh
                            po = 64 * (h % 2)
                            co = 1024 * (h // 2)
                            # bias preload; region matches the kq accumulate
                            nc.tensor.matmul(
                                ps[:, WIN * hh + c_lo:WIN * hh + c_hi],
                                i3, d3[:, :, WIN * hh + c_lo:WIN * hh + c_hi],
                                start=True, stop=False,
                                perf_mode=DR,
                            )
                            nc.tensor.matmul(
                                ps[:, WIN * hh + c_lo:WIN * hh + c_hi],
                                kT[po:po + 64, co + 128 * kt:co + 128 * (kt + 1)],
                                qT[po:po + 64, co + q_lo:co + q_hi],
                                start=False, stop=True,
                            )
                        nc.scalar.activation(
                            out=et[:, 4 * WIN * g:4 * WIN * (g + 1)].rearrange(
                                "p (h w) -> p h w", w=WIN)[:, :, c_lo:c_hi],
                            in_=ps.rearrange("p (h w) -> p h w", w=WIN)[:, :, c_lo:c_hi],
                            func=AF.Exp,
                            scale=EXP_SCALE,
                        )

                def av_block(qt):
                    for g in range(2):
                        # full-bank tile so the 260-col view never crosses
                        # a PSUM bank boundary
                        ops5 = avp.tile([128, 512], f32, tag="small_ps")
                        ops = ops5[:, 0:4 * 65]
                        for hh in range(4):
                            h = 4 * g + hh
                            o = 65 * hh
                            last = ("r" if qt + 1 < NLT else "l")
                            # diagonal k-tile: q-window cols [64, 192)
                            nc.tensor.matmul(
                                ops[:, o:o + 65],
                                expts[qt][:, WIN * h + 64:WIN * h + 192],
                                v_bf[:, 520 * qt + 65 * h:520 * qt + 65 * (h + 1)],
                                start=True, stop=False,
                                skip_group_check=True,
                            )
                            if qt >= 1:  # k-tile qt-1 covers q_local [0, 64)
                                nc.tensor.matmul(
                                    ops[0:64, o:o + 65],
                                    expts[qt - 1][:, WIN * h + 192:WIN * h + 256],
                                    v_bf[:, 520 * (qt - 1) + 65 * h:
                                         520 * (qt - 1) + 65 * (h + 1)],
                                    start=False, stop=(last == "l"),
                                    skip_group_check=True,
                                )
                            if qt + 1 < NLT:  # k-tile qt+1 covers [64, 128)
                                nc.tensor.matmul(
                                    ops[64:128, o:o + 65],
                                    expts[qt + 1][:, WIN * h:WIN * h + 64],
                                    v_bf[:, 520 * (qt + 1) + 65 * h:
                                         520 * (qt + 1) + 65 * (h + 1)],
                                    start=False, stop=(last == "r"),
                                    skip_group_check=True,
                                )
                        rec = small.tile([128, 4], f32, tag="rec")
                        nc.vector.reciprocal(
                            out=rec,
                            in_=ops.rearrange("p (h k) -> p h k", k=65)[:, :, 64:65],
                        )
                        sl = 512 * (qt % 2)
                        nc.vector.tensor_tensor(
                            out=o_norm[:, sl + 256 * g:
                                       sl + 256 * (g + 1)].rearrange(
                                "p (h d) -> p h d", d=64),
                            in0=ops.rearrange("p (h k) -> p h k", k=65)[:, :, 0:64],
                            in1=bcast64(rec),
                            op=ALU.mult,
                        )
                    sl = 512 * (qt % 2)
                    for j in range(NDC):
                        nc.sync.dma_start_transpose(
                            out=oT[:, 256 * j + 128 * (qt % 2):
                                   256 * j + 128 * (qt % 2) + 128],
                            in_=o_norm[:, sl + 128 * j:sl + 128 * (j + 1)],
                        )

                def out_proj(lt):
                    ps = avp.tile([128, 512], f32, tag="small_ps")
                    for c in range(NDC):
                        nc.tensor.matmul(
                            ps,
                            oT[:, 256 * c + 128 * (lt % 2):
                               256 * c + 128 * (lt % 2) + 128],
                            wo[:, 512 * c:512 * (c + 1)],
                            start=(c == 0), stop=(c == NDC - 1),
                        )
                    nc.vector.tensor_tensor(
                        out=resid_out_sb[:, 512 * lt:512 * (lt + 1)],
                        in0=ps,
                        in1=resid_in_sb[:, 512 * lt:512 * (lt + 1)],
                        op=ALU.add,
                    )

                for kt in range(NLT):
                    if kt == 2 and kt2_cb is not None:
                        kt2_cb(mtp)
                    scores_exp(kt)
                    if kt >= 1:
                        av_block(kt - 1)
                        out_proj(kt - 1)
                    if kt >= 2:
                        expts.pop(kt - 3, None)
                        tail_cb(kt - 2, mtp)
                    if kt == 5 and half0_cb is not None:
                        half0_cb()
                av_block(NLT - 1)
                out_proj(NLT - 1)
                tail_cb(NLT - 2, mtp)
                tail_cb(NLT - 1, mtp)

        def make_mov_tail(in_sb, mv, psum_tag="mov_ps"):
            """Returns (cb, mov_sb): cb(t, pool) emits the banded A @ in_sb
            matmuls, the bf16 drain, and LN stats for tile t."""
            ensure_a()
            mov_sb = movp.tile([128, NLT * 512], bf16, tag="mov")

            def cb(t, pool):
                ps = pool.tile([128, 512], f32, tag=psum_tag)
                js = [j for j in (t - 1, t, t + 1) if 0 <= j < NLT]
                for ji, j in enumerate(js):
                    bi = a_blocks[(t, j)]
                    nc.tensor.matmul(
                        ps,
                        a_sb[:, 128 * bi:128 * (bi + 1)],
                        in_sb[:, 512 * j:512 * (j + 1)],
                        start=(ji == 0), stop=(ji == len(js) - 1),
                    )
                if t % 2 == 0 or t >= 6:
                    nc.scalar.copy(out=mov_sb[:, 512 * t:512 * (t + 1)],
                                   in_=ps)
                else:
                    nc.vector.tensor_copy(out=mov_sb[:, 512 * t:512 * (t + 1)],
                                          in_=ps)
                st6 = small.tile([128, 6], f32, tag="st6")
                if t >= 6:  # last tiles: stats from PSUM, parallel with drain
                    nc.vector.bn_stats(out=st6, in_=ps)
                else:
                    nc.vector.bn_stats(out=st6,
                                       in_=mov_sb[:, 512 * t:512 * (t + 1)])
                nc.vector.bn_aggr(out=mv[:, 2 * t:2 * (t + 1)], in_=st6)

            return cb, mov_sb

        def make_ln_half(mov_sb, mv, n_bf=None, nT=None, nT8=None, q8scale=SX,
                         n_sb=None, out_dma=None, bf_scale=1.0, eps=EPS):
            """Returns (main_cb(half), quant_cb(half)). main: rstd via Act
            Sqrt + DVE reciprocal, then per-tile normalize (DVE 4x) +
            SBUF->SBUF transposes / output DMA. quant: one Act op per half
            (bf16 -> fp8), emitted separately so the caller controls the
            Act queue order."""
            mv3 = mv.rearrange("p (t two) -> p t two", two=2)
            rstds = {}

            def main_cb(half, nbf_eng=None, t0=None, nt=4):
                t0 = 4 * half if t0 is None else t0
                # rstd = 1/sqrt(var+eps): quake seed + 1 Newton step, all on
                # DVE (an Act Sqrt would thrash the activation table between
                # the surrounding Exp/Gelu ops, costing ~1.3us per reload)
                vv_t = stats_p.tile([128, 4], f32, tag="vv")
                vv = vv_t[:, :nt]
                nc.vector.tensor_scalar_add(out=vv, in0=mv3[:, t0:t0 + nt, 1:2],
                                            scalar1=eps)
                rstd_t = stats_p.tile([128, 4], f32, tag="rstd")
                rstd = rstd_t[:, :nt]
                yi = rstd.bitcast(mybir.dt.int32)
                nc.vector.tensor_scalar(
                    out=yi, in0=vv.bitcast(mybir.dt.int32),
                    scalar1=1, scalar2=None, op0=ALU.arith_shift_right,
                )
                nc.vector.tensor_scalar(
                    out=yi, in0=yi, scalar1=-1, scalar2=0x5F3759DF,
                    op0=ALU.mult, op1=ALU.add,
                )
                t1_t = stats_p.tile([128, 4], f32, tag="t1")
                t1 = t1_t[:, :nt]
                nc.vector.tensor_tensor(out=t1, in0=rstd, in1=rstd, op=ALU.mult)
                nc.vector.tensor_tensor(out=t1, in0=t1, in1=vv, op=ALU.mult)
                nc.vector.tensor_scalar(
                    out=t1, in0=t1, scalar1=-0.5, scalar2=1.5,
                    op0=ALU.mult, op1=ALU.add,
                )
                nc.vector.tensor_tensor(out=rstd, in0=rstd, in1=t1,
                                        op=ALU.mult)
                rstds[half] = rstd
                rstd_bf = rstd
                if bf_scale != 1.0:
                    rstd_bf_t = stats_p.tile([128, 4], f32, tag="rstd_bf")
                    rstd_bf = rstd_bf_t[:, :nt]
                    nc.vector.tensor_scalar_mul(out=rstd_bf, in0=rstd,
                                                scalar1=bf_scale)
                for tt in range(nt):
                    t = t0 + tt
                    dst = n_bf if n_bf is not None else n_sb
                    (nbf_eng or nc.vector).tensor_scalar(
                        out=dst[:, 512 * t:512 * (t + 1)],
                        in0=mov_sb[:, 512 * t:512 * (t + 1)],
                        scalar1=mv[:, 2 * t:2 * t + 1],
                        scalar2=rstd_bf[:, tt:tt + 1],
                        op0=ALU.subtract,
                        op1=ALU.mult,
                    )
                    if nT is not None:
                        for j in range(NDC):
                            nc.sync.dma_start_transpose(
                                out=nT[:, 1024 * j + 128 * t:
                                       1024 * j + 128 * (t + 1)],
                                in_=n_bf[:, 512 * t + 128 * j:
                                         512 * t + 128 * (j + 1)],
                            )
                    if out_dma is not None:
                        deng = nc.sync if t % 2 == 0 else nc.scalar
                        deng.dma_start(
                            out=out_dma[128 * t:128 * (t + 1), :],
                            in_=n_sb[:, 512 * t:512 * (t + 1)],
                        )

            def quant_cb(half, split=False, pool=False):
                nr = nT.rearrange("p (c l) -> p c l", c=NDC)
                n8r = nT8.rearrange("p (c l) -> p c l", c=NDC)
                sl = slice(512 * half, 512 * (half + 1))
                if pool:  # idle GpSimd: off the loop-saturated DVE/Act
                    nc.gpsimd.tensor_scalar_mul(
                        out=n8r[:, :, sl], in0=nr[:, :, sl], scalar1=q8scale,
                    )
                elif split:  # halve latency: chunks 0-1 on DVE, 2-3 on Act
                    nc.vector.tensor_scalar_mul(
                        out=n8r[:, 0:2, sl], in0=nr[:, 0:2, sl],
                        scalar1=q8scale,
                    )
                    nc.scalar.activation(
                        out=n8r[:, 2:4, sl], in_=nr[:, 2:4, sl],
                        func=AF.Identity, scale=q8scale,
                    )
                else:
                    nc.scalar.activation(
                        out=n8r[:, :, sl], in_=nr[:, :, sl],
                        func=AF.Identity, scale=q8scale,
                    )

            return main_cb, quant_cb

        def projection_q8_dr(w8, srcT8, qT, lh, pool=None):
            """Q projection (fp8 src) for one l-half; drains split DVE/Act."""
            wr = w8.rearrange("p (c n) -> p c n", c=NDC)
            sr = srcT8.rearrange("p (c l) -> p c l", c=NDC)

            def emit(qp, tag):
                for t in range(NDC):
                    ps = qp.tile([128, 512], f32, tag=tag)
                    for c2 in range(2):
                        nc.tensor.matmul(
                            ps,
                            wr[:, 2 * c2:2 * c2 + 2, 128 * t:128 * (t + 1)],
                            sr[:, 2 * c2:2 * c2 + 2, 512 * lh:512 * (lh + 1)],
                            start=(c2 == 0), stop=(c2 == 1),
                            perf_mode=DR,
                        )
                    dsl = qT[:, 1024 * t + 512 * lh:1024 * t + 512 * (lh + 1)]
                    if t % 2 == 0:
                        nc.vector.tensor_scalar_mul(out=dsl, in0=ps,
                                                    scalar1=DSC)
                    else:
                        nc.scalar.activation(out=dsl, in_=ps, func=AF.Identity,
                                             scale=DSC)

            if pool is not None:
                emit(pool, "mov_ps")
            else:
                with tc.tile_pool(name="q_ps_pool", space="PSUM", bufs=2) as qp:
                    emit(qp, "proj_ps")

        # ================= the layer =================
        for _rep in range(reps):
            # DMA: SP queue carries xT8/x_sb + tiny constants (then stays
            # free for the in-loop transposes); the gpsimd SWDGE queue
            # carries all weights, ordered by first use.
            xT8 = load_srcT8(xT8_d, "srcT8")
            with tc.tile_pool(name="attn_w", bufs=1) as wpool:
                wk1 = load_w8(wpool, wk_sa8, "wk8", eng=nc.gpsimd)
                wq1 = load_w8(wpool, wq_sa8, "wq8", eng=nc.gpsimd)
                wv1 = load_w8(wpool, wv_sa8, "wv8", eng=nc.gpsimd)
                x_sb = bfbuf.tile([128, NLT * 512], bf16, tag="x_sb")
                nc.sync.dma_start(
                    out=x_sb.rearrange("p (t d) -> p t d", t=NLT),
                    in_=x_bf_d.rearrange("(t p) d -> p t d", p=128),
                )
                encT8 = load_srcT8(encT8_d, "srcT8", eng=nc.gpsimd)
                wk2 = load_w8(wpool, wk_ca8, "wk8_ca", eng=nc.gpsimd)
                wv2 = load_w8(wpool, wv_ca8, "wv8_ca", eng=nc.gpsimd)
                wo1 = load_w16(wpool, wo_sa, "wo", eng=nc.gpsimd)
                ensure_a()
                wq2 = load_w8(wpool, wq_ca8, "wq8_ca", eng=nc.gpsimd)
                wo2 = load_w16(wpool, wo_ca, "wo_ca", eng=nc.gpsimd)
                w1 = wpool.tile([128, NDC * DFF], fp8, tag="w1")
                nc.gpsimd.dma_start(
                    out=w1.rearrange("p (c n) -> p c n", c=NDC),
                    in_=w18.rearrange("(c p) n -> p c n", p=128),
                )
                w2 = wpool.tile([128, NFT * 512], fp8, tag="w2")
                nc.gpsimd.dma_start(
                    out=w2.rearrange("p (c n) -> p c n", c=NFT),
                    in_=w28.rearrange("(c p) n -> p c n", p=128),
                )

                # --- ALL k/v projections (SA + CA) share one PSUM scope so
                # the SA->CA boundary only has the q2 projections left ---
                kT1 = bfbuf.tile([128, NDC * 1024], bf16, tag="kT")
                qT1 = bfbuf.tile([128, NDC * 1024], bf16, tag="qT")
                kT2 = bfbuf.tile([128, NDC * 1024], bf16, tag="kT2")
                v1 = make_v_tile("v_bf")
                v2 = make_v_tile("v_bf2")
                qT2 = bfbuf.tile([128, NDC * 1024], bf16, tag="qT")
                with tc.tile_pool(name="kv_ps", space="PSUM", bufs=3) as kvp, \
                     tc.tile_pool(name="kv_v", space="PSUM", bufs=2) as kvv:
                    projection_T_dr(wk1, xT8, kT1, kvp, "dve")
                    projection_T_dr(wq1, xT8, qT1, kvp, "act")
                    projection_nat_v_dr(wv1, xT8, v1, kvv, "dve")
                    projection_T_dr(wk2, encT8, kT2, kvp,
                                    lambda t: "dve" if t < 2 else "act")
                    projection_nat_v_dr(wv2, encT8, v2, kvv,
                                        lambda lt: "dve" if lt % 2 else "act")

                # --- self attention + residual (mov1+stats in the loop) ---
                r1 = streams.tile([128, NLT * 512], f32r, tag="stream")
                mv1 = stats_p.tile([128, NLT * 2], f32, tag="mv")
                mov1_cb, mov1 = make_mov_tail(r1, mv1)
                n1_bf = bfbuf.tile([128, NLT * 512], bf16, tag="n_bf")
                n1T = srcp16.tile([128, NDC * 1024], bf16, tag="srcT")
                n1T8 = bfbuf.tile([128, NDC * 1024], fp8, tag="n1T8")
                ln1_main, ln1_quant = make_ln_half(mov1, mv1, n_bf=n1_bf,
                                                   nT=n1T, nT8=n1T8)

                def ln1_h0():
                    ln1_main(0, nbf_eng=nc.gpsimd)
                    ln1_quant(0, pool=True)

                attention_loop(wo1, kT1, qT1, v1, x_sb, r1,
                               tail_cb=mov1_cb, half0_cb=ln1_h0)
                ln1_main(1)

                # --- CA q projection lh=0 only; lh=1 (and its h1 quant) are
                # injected into the CA loop at kt==2 — CA scores for kt<=2
                # read q columns < 512, so the loop starts immediately ---
                projection_q8_dr(wq2, n1T8, qT2, 0)

                def ca_kt2(pool):
                    ln1_quant(1, split=True)
                    projection_q8_dr(wq2, n1T8, qT2, 1, pool=pool)

                # --- cross attention + residual (mov2+stats in the loop) ---
                r2 = streams.tile([128, NLT * 512], f32r, tag="stream")
                mv2 = stats_p.tile([128, NLT * 2], f32, tag="mv")
                mov2_cb, mov2 = make_mov_tail(r2, mv2)
                n2_bf = bfbuf.tile([128, NLT * 512], bf16, tag="n_bf")
                n2T = srcp16.tile([128, NDC * 1024], bf16, tag="srcT")
                # shares n1T8's buffer: n1T8's last read (CA q lh=1) is long
                # before the first n2T8 write (CA kt==5 quant)
                n2T8 = bfbuf.tile([128, NDC * 1024], fp8, tag="n1T8")
                # n2_bf carries x SW2 (the FFN residual base); n2T8 is x SX.
                ln2_main, ln2_quant = make_ln_half(mov2, mv2, n_bf=n2_bf,
                                                   nT=n2T, nT8=n2T8,
                                                   q8scale=SX / SW2,
                                                   bf_scale=SW2)

                def ln2_h0():
                    ln2_main(0, nbf_eng=nc.gpsimd)
                    ln2_quant(0, pool=True)

                attention_loop(wo2, kT2, qT2, v2, n1_bf, r2,
                               tail_cb=mov2_cb, half0_cb=ln2_h0,
                               kt2_cb=ca_kt2)
                ln2_main(1)

                # --- FFN (fp8 DoubleRow; r3 carries x SW2, LN3 absorbs it) ---
                r3 = streams.tile([128, NLT * 512], f32r, tag="stream")
                mv3 = stats_p.tile([128, NLT * 2], f32, tag="mv")
                mov3_cb, mov3 = make_mov_tail(r3, mv3, psum_tag="ff2_ps")
                out_sb = streams.tile([128, NLT * 512], bf16, tag="stream")
                ln3_main, _ = make_ln_half(mov3, mv3, n_sb=out_sb,
                                           out_dma=out_d, eps=EPS * SW2 * SW2)
                with tc.tile_pool(name="h_psp", space="PSUM", bufs=2) as hps, \
                     tc.tile_pool(name="ff2_psp", space="PSUM", bufs=3) as f2ps:
                    w1r = w1.rearrange("p (c n) -> p c n", c=NDC)
                    w2r = w2.rearrange("p (c n) -> p c n", c=NFT)
                    n2r = n2T8.rearrange("p (c l) -> p c l", c=NDC)
                    g1T_a = bfbuf.tile([128, NFT * 512], fp8, tag="g1T")
                    g1T_b = bfbuf.tile([128, NFT * 512], fp8, tag="g1T_b")
                    g1Ts = [g1T_a, g1T_b]

                    def ffn1_f2(lh, f2):
                        ps = hps.tile([128, 1024], f32, tag="h_ps")
                        for fh in range(2):
                            f = 2 * f2 + fh
                            for c2 in range(2):
                                nc.tensor.matmul(
                                    ps[:, 512 * fh:512 * (fh + 1)],
                                    w1r[:, 2 * c2:2 * c2 + 2,
                                        128 * f:128 * (f + 1)],
                                    n2r[:, 2 * c2:2 * c2 + 2,
                                        512 * lh:512 * (lh + 1)],
                                    start=(c2 == 0), stop=(c2 == 1),
                                    perf_mode=DR,
                                )
                        nc.scalar.activation(
                            out=g1Ts[lh][:, 1024 * f2:1024 * (f2 + 1)],
                            in_=ps, func=AF.Gelu, scale=1.0 / (SX * SW),
                        )

                    def ffn2_tile(lh, ltt):
                        lt = 4 * lh + ltt
                        g1r = g1Ts[lh].rearrange("p (c n) -> p c n", c=NFT)
                        ps = f2ps.tile([128, 512], f32, tag="ff2_ps")
                        for c2 in range(NFT // 2):
                            nc.tensor.matmul(
                                ps,
                                g1r[:, 2 * c2:2 * c2 + 2,
                                    128 * ltt:128 * (ltt + 1)],
                                w2r[:, 2 * c2:2 * c2 + 2, :],
                                start=(c2 == 0), stop=(c2 == NFT // 2 - 1),
                                perf_mode=DR,
                            )
                        nc.vector.tensor_tensor(
                            out=r3[:, 512 * lt:512 * (lt + 1)],
                            in0=ps,
                            in1=n2_bf[:, 512 * lt:512 * (lt + 1)],
                            op=ALU.add,
                        )

                    # lh0 FFN1 (h0's LN chain starts at kt==5 and its
                    # quant runs on Pool, so it is ready first)
                    for f2 in range(NFT // 2):
                        ffn1_f2(0, f2)
                    # lh1 FFN1 interleaved with lh0 FFN2 (fills PE idle)
                    ln2_quant(1, split=True)
                    for f2 in range(NFT // 2):
                        ffn1_f2(1, f2)
                        if f2 % 2 == 1:
                            ltt = f2 // 2
                            ffn2_tile(0, ltt)
                            if ltt >= 1:
                                mov3_cb(ltt - 1, f2ps)
                            if ltt == 2:
                                ln3_main(0, t0=0, nt=2)
                    for ltt in range(4):
                        lt = 4 + ltt
                        ffn2_tile(1, ltt)
                        mov3_cb(lt - 1, f2ps)
                        if lt == 5:
                            ln3_main(1, t0=2, nt=2)
                        if lt == 7:
                            ln3_main(2, t0=4, nt=2)
                    mov3_cb(NLT - 1, f2ps)
                    ln3_main(3, t0=6, nt=2)

    nc.compile()
    _CACHE[key] = nc
    return nc


def _make_in_maps(inputs):
    d_cat8, a_strip = _host_constants()

    def T(w):
        return np.ascontiguousarray(np.asarray(w, dtype=np.float32).T)

    def T8(w, s):
        return (T(w) * s).astype(F8)

    ident8 = np.concatenate([np.eye(128, dtype=np.float32)] * 2, axis=1)
    common = {
        "wq_sa8": T8(inputs["sa_Wq"], SW),
        "wk_sa8": T8(inputs["sa_Wk"], SW),
        "wv_sa8": T8(inputs["sa_Wv"], SW),
        "wo_sa": T(inputs["sa_Wo"]).astype(BF16),
        "wq_ca8": T8(inputs["ca_Wq"], SW),
        "wk_ca8": T8(inputs["ca_Wk"], SW),
        "wv_ca8": T8(inputs["ca_Wv"], SW),
        "wo_ca": T(inputs["ca_Wo"]).astype(BF16),
        "w18": T8(inputs["ff_W1"], SW),
        "w28": T8(inputs["ff_W2"], SW2),
        "d_cat8": d_cat8,
        "a_strip": a_strip,
        "ident8": ident8.astype(F8),
    }
    x = np.asarray(inputs["x"], dtype=np.float32)
    enc = np.asarray(inputs["enc_out"], dtype=np.float32)
    maps = []
    for b in range(B):
        m = dict(common)
        m["xT8"] = np.ascontiguousarray(x[b].T * SX).astype(F8)
        m["encT8"] = np.ascontiguousarray(enc[b].T * SX).astype(F8)
        m["x_bf"] = np.ascontiguousarray(x[b]).astype(BF16)
        maps.append(m)
    return maps


def kernel(**inputs):
    from concourse.bass_utils import run_bass_kernel_spmd

    nc = _build_program()
    in_maps = _make_in_maps(inputs)
    res = run_bass_kernel_spmd(nc, in_maps, list(range(B)))
    _CACHE["last_results"] = res
    out = np.stack([np.asarray(res.results[b]["out"]) for b in range(B)])
    return out.astype(np.float32)
